# revision 7
# baseline (speedup 1.0000x reference)
"""MoE gating kernel for Trainium2 (Bass/Tile), 8-core data parallel. V2.

Structure: Wt-resident gate matmul (moving = tokens, N=512) to minimize
PE instruction count for the f32 matmul:

per core (2048 tokens), per token-group of 512 (4 tiles of 128):
  - DMA 4 x-tiles [128, 2048] (natural layout)
  - PE-transpose every [128,128] block -> PSUM -> copy to xT [128, 16, 512]
  - 16 accumulating matmuls: lhsT = Wt chunk [128h, 8e] (tiny stationary),
    rhs = xT[:, c, :] [128h, 512t] -> PSUM gateT [8, 512]
  - copy gateT -> SBUF, 4 tiny PE back-transposes -> [128, 8] per tile
  - DVE top-2-of-8: +b, max (sorts desc), max_index, mask >= 2nd max,
    sparse = where(mask, logits, -inf)
  - DMA out gate/sparse/idx tiles
"""

import sys

for _p in ("/opt/trn_rl_repo", "/root/.axon_site"):
    if _p not in sys.path:
        sys.path.insert(0, _p)

from contextlib import ExitStack

import numpy as np

import concourse.bacc as bacc
import concourse.mybir as mybir
import concourse.tile as tile
from concourse.bass_utils import run_bass_kernel_spmd

B, S, H, E, TOPK = 4, 4096, 2048, 8, 2
N_CORES = 8
TOKENS = B * S
TOK_PER_CORE = TOKENS // N_CORES  # 2048
P = 128
N_TILES = TOK_PER_CORE // P       # 16
HC = H // P                       # 16 h-chunks
GRP = 512                         # tokens per matmul group
N_GRP = TOK_PER_CORE // GRP       # 4
TPG = GRP // P                    # tiles per group = 4
F32 = mybir.dt.float32
NEG_INF = float("-inf")

_cache = {}
_EYE = np.eye(P, dtype=np.float32)


def _build():
    nc = bacc.Bacc("TRN2", target_bir_lowering=False, debug=False)

    x_d = nc.dram_tensor("x", [TOK_PER_CORE, H], F32, kind="ExternalInput")
    wt_d = nc.dram_tensor("wt", [P, HC * E], F32, kind="ExternalInput")
    bb_d = nc.dram_tensor("bb", [P, E], F32, kind="ExternalInput")
    id_d = nc.dram_tensor("ident", [P, P], F32, kind="ExternalInput")
    gate_d = nc.dram_tensor("gate", [TOK_PER_CORE, E], F32, kind="ExternalOutput")
    sp_d = nc.dram_tensor("sparse", [TOK_PER_CORE, E], F32, kind="ExternalOutput")
    idx_d = nc.dram_tensor("idx", [TOK_PER_CORE, TOPK], mybir.dt.int32,
                           kind="ExternalOutput")

    with tile.TileContext(nc) as tc:
        with ExitStack() as ctx:
            consts = ctx.enter_context(tc.tile_pool(name="consts", bufs=1))
            xpool = ctx.enter_context(tc.tile_pool(name="xin", bufs=3))
            xtg = ctx.enter_context(tc.tile_pool(name="xtg", bufs=3))
            trps = ctx.enter_context(
                tc.tile_pool(name="trps", bufs=3, space="PSUM"))
            gtps = ctx.enter_context(
                tc.tile_pool(name="gtps", bufs=2, space="PSUM"))
            btps = ctx.enter_context(
                tc.tile_pool(name="btps", bufs=2, space="PSUM"))
            gts_pool = ctx.enter_context(tc.tile_pool(name="gts", bufs=2))
            spool = ctx.enter_context(tc.tile_pool(name="small", bufs=6))

            wt_sb = consts.tile([P, HC * E], F32)
            nc.sync.dma_start(wt_sb[:], wt_d[:, :])
            bb_sb = consts.tile([P, E], F32)
            nc.sync.dma_start(bb_sb[:], bb_d[:, :])
            ident = consts.tile([P, P], F32)
            nc.sync.dma_start(ident[:], id_d[:, :])

            ncopy = 0

            def load_transpose(g):
                xT = xtg.tile([P, HC, GRP], F32, tag="xtg")
                nonlocal ncopy
                for t4 in range(TPG):
                    tt = g * TPG + t4
                    xt = xpool.tile([P, H], F32, tag="xin")
                    nc.sync.dma_start(xt[:], x_d[tt * P:(tt + 1) * P, :])
                    for gg in range(4):
                        ps = trps.tile([P, 4, P], F32, tag="trp")
                        for j in range(4):
                            c = gg * 4 + j
                            nc.tensor.transpose(
                                ps[:, j], xt[:, c * P:(c + 1) * P], ident[:])
                        dst = xT[:, gg * 4:(gg + 1) * 4, t4 * P:(t4 + 1) * P]
                        if ncopy % 2 == 0:
                            nc.vector.tensor_copy(dst, ps[:])
                        else:
                            nc.scalar.copy(dst, ps[:])
                        ncopy += 1
                return xT

            xTs = {0: load_transpose(0)}
            for g in range(N_GRP):
                if g + 1 < N_GRP:
                    xTs[g + 1] = load_transpose(g + 1)
                xT = xTs.pop(g)
                gT = gtps.tile([E, GRP], F32, tag="gt")
                for c in range(HC):
                    nc.tensor.matmul(
                        gT[:],
                        wt_sb[:, c * E:(c + 1) * E],
                        xT[:, c, :],
                        start=(c == 0),
                        stop=(c == HC - 1),
                    )
                gts = gts_pool.tile([E, GRP], F32, tag="gts")
                nc.vector.tensor_copy(gts[:], gT[:])

                for t4 in range(TPG):
                    tt = g * TPG + t4
                    bt = btps.tile([P, E], F32, tag="bt")
                    nc.tensor.transpose(
                        bt[:], gts[:, t4 * P:(t4 + 1) * P], ident[0:E, 0:E])

                    gate = spool.tile([P, E], F32, tag="gate")
                    nc.vector.tensor_add(gate[:], bt[:], bb_sb[:])

                    mx8 = spool.tile([P, 8], F32, tag="mx8")
                    nc.vector.max(out=mx8[:], in_=gate[:])
                    ix8 = spool.tile([P, 8], mybir.dt.uint32, tag="ix8")
                    nc.vector.max_index(out=ix8[:], in_max=mx8[:],
                                        in_values=gate[:])
                    ix2 = spool.tile([P, TOPK], mybir.dt.int32, tag="ix2")
                    nc.vector.tensor_copy(ix2[:], ix8[:, 0:TOPK])

                    keep = spool.tile([P, E], mybir.dt.uint32, tag="keep")
                    nc.vector.tensor_scalar(
                        out=keep[:], in0=gate[:], scalar1=mx8[:, 1:2],
                        scalar2=None, op0=mybir.AluOpType.is_ge)
                    sp = spool.tile([P, E], F32, tag="sp")
                    nc.vector.memset(sp[:], NEG_INF)
                    nc.vector.copy_predicated(sp[:], keep[:], gate[:])

                    nc.scalar.dma_start(gate_d[tt * P:(tt + 1) * P, :], gate[:])
                    nc.scalar.dma_start(sp_d[tt * P:(tt + 1) * P, :], sp[:])
                    nc.scalar.dma_start(idx_d[tt * P:(tt + 1) * P, :], ix2[:])

    nc.compile()
    return nc


def _get_nc():
    if "nc" not in _cache:
        _cache["nc"] = _build()
    return _cache["nc"]


def _make_in_maps(x, W, b):
    x = np.ascontiguousarray(np.asarray(x, dtype=np.float32)).reshape(TOKENS, H)
    W = np.asarray(W, dtype=np.float32)
    b = np.asarray(b, dtype=np.float32)

    # Wt packed: [p, c*8+e] = W[e, c*128+p]
    wt = np.ascontiguousarray(
        W.T.reshape(HC, P, E).transpose(1, 0, 2).reshape(P, HC * E))
    bb = np.ascontiguousarray(np.broadcast_to(b, (P, E)))

    return [
        {"x": x[i * TOK_PER_CORE:(i + 1) * TOK_PER_CORE], "wt": wt, "bb": bb,
         "ident": _EYE}
        for i in range(N_CORES)
    ]


def kernel(x: np.ndarray, W: np.ndarray, b: np.ndarray):
    in_maps = _make_in_maps(x, W, b)
    nc = _get_nc()
    res = run_bass_kernel_spmd(nc, in_maps, list(range(N_CORES)))

    gate = np.concatenate([r["gate"] for r in res.results], axis=0)
    sparse = np.concatenate([r["sparse"] for r in res.results], axis=0)
    idx = np.concatenate([r["idx"] for r in res.results], axis=0)

    return (
        sparse.reshape(B, S, E),
        idx.reshape(B, S, TOPK).astype(np.int32),
        gate.reshape(TOKENS, E),
    )


if __name__ == "__main__":
    rng = np.random.default_rng(0)
    x = rng.standard_normal((B, S, H), dtype=np.float32)
    W = (rng.standard_normal((E, H), dtype=np.float32) / np.sqrt(H)).astype(
        np.float32)
    b = np.zeros((E,), dtype=np.float32)
    sp, ix, gl = kernel(x=x, W=W, b=b)
    print("shapes:", sp.shape, ix.shape, gl.shape, sp.dtype, ix.dtype, gl.dtype)


# revision 8
# speedup vs baseline: 1.0607x; 1.0607x over previous
"""MoE gating kernel for Trainium2 (Bass/Tile), 8-core data parallel. V2.

Structure: Wt-resident gate matmul (moving = tokens, N=512) to minimize
PE instruction count for the f32 matmul:

per core (2048 tokens), per token-group of 512 (4 tiles of 128):
  - DMA 4 x-tiles [128, 2048] (natural layout)
  - PE-transpose every [128,128] block -> PSUM -> copy to xT [128, 16, 512]
  - 16 accumulating matmuls: lhsT = Wt chunk [128h, 8e] (tiny stationary),
    rhs = xT[:, c, :] [128h, 512t] -> PSUM gateT [8, 512]
  - copy gateT -> SBUF, 4 tiny PE back-transposes -> [128, 8] per tile
  - DVE top-2-of-8: +b, max (sorts desc), max_index, mask >= 2nd max,
    sparse = where(mask, logits, -inf)
  - DMA out gate/sparse/idx tiles
"""

import sys

for _p in ("/opt/trn_rl_repo", "/root/.axon_site"):
    if _p not in sys.path:
        sys.path.insert(0, _p)

from contextlib import ExitStack

import numpy as np

import concourse.bacc as bacc
import concourse.mybir as mybir
import concourse.tile as tile
from concourse.bass_utils import run_bass_kernel_spmd

B, S, H, E, TOPK = 4, 4096, 2048, 8, 2
N_CORES = 8
TOKENS = B * S
TOK_PER_CORE = TOKENS // N_CORES  # 2048
P = 128
N_TILES = TOK_PER_CORE // P       # 16
HC = H // P                       # 16 h-chunks
GRP = 512                         # tokens per matmul group
N_GRP = TOK_PER_CORE // GRP       # 4
TPG = GRP // P                    # tiles per group = 4
F32 = mybir.dt.float32
NEG_INF = float("-inf")

_cache = {}
_EYE = np.eye(P, dtype=np.float32)


def _build():
    nc = bacc.Bacc("TRN2", target_bir_lowering=False, debug=False)

    x_d = nc.dram_tensor("x", [TOK_PER_CORE, H], F32, kind="ExternalInput")
    wt_d = nc.dram_tensor("wt", [P, HC * E], F32, kind="ExternalInput")
    bb_d = nc.dram_tensor("bb", [P, E], F32, kind="ExternalInput")
    id_d = nc.dram_tensor("ident", [P, P], F32, kind="ExternalInput")
    gate_d = nc.dram_tensor("gate", [TOK_PER_CORE, E], F32, kind="ExternalOutput")
    sp_d = nc.dram_tensor("sparse", [TOK_PER_CORE, E], F32, kind="ExternalOutput")
    idx_d = nc.dram_tensor("idx", [TOK_PER_CORE, TOPK], mybir.dt.int32,
                           kind="ExternalOutput")

    with tile.TileContext(nc) as tc:
        with ExitStack() as ctx:
            consts = ctx.enter_context(tc.tile_pool(name="consts", bufs=1))
            xpool = ctx.enter_context(tc.tile_pool(name="xin", bufs=6))
            xtg = ctx.enter_context(tc.tile_pool(name="xtg", bufs=3))
            trps = ctx.enter_context(
                tc.tile_pool(name="trps", bufs=4, space="PSUM"))
            gtps = ctx.enter_context(
                tc.tile_pool(name="gtps", bufs=2, space="PSUM"))
            btps = ctx.enter_context(
                tc.tile_pool(name="btps", bufs=2, space="PSUM"))
            gts_pool = ctx.enter_context(tc.tile_pool(name="gts", bufs=2))
            spool = ctx.enter_context(tc.tile_pool(name="small", bufs=6))

            ident = consts.tile([P, P], F32)
            nc.sync.dma_start(ident[:], id_d[:, :])
            wt_sb = consts.tile([P, HC * E], F32)
            nc.sync.dma_start(wt_sb[:], wt_d[:, :])
            bb_sb = consts.tile([P, E], F32)
            nc.sync.dma_start(bb_sb[:], bb_d[:, :])

            ncopy = 0

            def load_transpose(g):
                xT = xtg.tile([P, HC, GRP], F32, tag="xtg")
                nonlocal ncopy
                for t4 in range(TPG):
                    tt = g * TPG + t4
                    xh = []
                    for h in range(2):
                        xt = xpool.tile([P, H // 2], F32, tag="xin")
                        nc.sync.dma_start(
                            xt[:], x_d[tt * P:(tt + 1) * P,
                                       h * (H // 2):(h + 1) * (H // 2)])
                        xh.append(xt)
                    for gg in range(4):
                        ps = trps.tile([P, 4, P], F32, tag="trp")
                        for j in range(4):
                            c = gg * 4 + j
                            nc.tensor.transpose(
                                ps[:, j],
                                xh[c // 8][:, (c % 8) * P:(c % 8 + 1) * P],
                                ident[:])
                        dst = xT[:, gg * 4:(gg + 1) * 4, t4 * P:(t4 + 1) * P]
                        if ncopy % 2 == 0:
                            nc.vector.tensor_copy(dst, ps[:])
                        else:
                            nc.scalar.copy(dst, ps[:])
                        ncopy += 1
                return xT

            xTs = {0: load_transpose(0)}
            for g in range(N_GRP):
                if g + 1 < N_GRP:
                    xTs[g + 1] = load_transpose(g + 1)
                xT = xTs.pop(g)
                gT = gtps.tile([E, GRP], F32, tag="gt")
                for c in range(HC):
                    nc.tensor.matmul(
                        gT[:],
                        wt_sb[:, c * E:(c + 1) * E],
                        xT[:, c, :],
                        start=(c == 0),
                        stop=(c == HC - 1),
                    )
                gts = gts_pool.tile([E, GRP], F32, tag="gts")
                nc.vector.tensor_copy(gts[:], gT[:])

                for t4 in range(TPG):
                    tt = g * TPG + t4
                    bt = btps.tile([P, E], F32, tag="bt")
                    nc.tensor.transpose(
                        bt[:], gts[:, t4 * P:(t4 + 1) * P], ident[0:E, 0:E])

                    gate = spool.tile([P, E], F32, tag="gate")
                    nc.vector.tensor_add(gate[:], bt[:], bb_sb[:])

                    mx8 = spool.tile([P, 8], F32, tag="mx8")
                    nc.vector.max(out=mx8[:], in_=gate[:])
                    ix8 = spool.tile([P, 8], mybir.dt.uint32, tag="ix8")
                    nc.vector.max_index(out=ix8[:], in_max=mx8[:],
                                        in_values=gate[:])
                    ix2 = spool.tile([P, TOPK], mybir.dt.int32, tag="ix2")
                    nc.vector.tensor_copy(ix2[:], ix8[:, 0:TOPK])

                    keep = spool.tile([P, E], mybir.dt.uint32, tag="keep")
                    nc.vector.tensor_scalar(
                        out=keep[:], in0=gate[:], scalar1=mx8[:, 1:2],
                        scalar2=None, op0=mybir.AluOpType.is_ge)
                    sp = spool.tile([P, E], F32, tag="sp")
                    nc.vector.memset(sp[:], NEG_INF)
                    nc.vector.copy_predicated(sp[:], keep[:], gate[:])

                    nc.scalar.dma_start(gate_d[tt * P:(tt + 1) * P, :], gate[:])
                    nc.scalar.dma_start(sp_d[tt * P:(tt + 1) * P, :], sp[:])
                    nc.scalar.dma_start(idx_d[tt * P:(tt + 1) * P, :], ix2[:])

    nc.compile()
    return nc


def _get_nc():
    if "nc" not in _cache:
        _cache["nc"] = _build()
    return _cache["nc"]


def _make_in_maps(x, W, b):
    x = np.ascontiguousarray(np.asarray(x, dtype=np.float32)).reshape(TOKENS, H)
    W = np.asarray(W, dtype=np.float32)
    b = np.asarray(b, dtype=np.float32)

    # Wt packed: [p, c*8+e] = W[e, c*128+p]
    wt = np.ascontiguousarray(
        W.T.reshape(HC, P, E).transpose(1, 0, 2).reshape(P, HC * E))
    bb = np.ascontiguousarray(np.broadcast_to(b, (P, E)))

    return [
        {"x": x[i * TOK_PER_CORE:(i + 1) * TOK_PER_CORE], "wt": wt, "bb": bb,
         "ident": _EYE}
        for i in range(N_CORES)
    ]


def kernel(x: np.ndarray, W: np.ndarray, b: np.ndarray):
    in_maps = _make_in_maps(x, W, b)
    nc = _get_nc()
    res = run_bass_kernel_spmd(nc, in_maps, list(range(N_CORES)))

    gate = np.concatenate([r["gate"] for r in res.results], axis=0)
    sparse = np.concatenate([r["sparse"] for r in res.results], axis=0)
    idx = np.concatenate([r["idx"] for r in res.results], axis=0)

    return (
        sparse.reshape(B, S, E),
        idx.reshape(B, S, TOPK).astype(np.int32),
        gate.reshape(TOKENS, E),
    )


if __name__ == "__main__":
    rng = np.random.default_rng(0)
    x = rng.standard_normal((B, S, H), dtype=np.float32)
    W = (rng.standard_normal((E, H), dtype=np.float32) / np.sqrt(H)).astype(
        np.float32)
    b = np.zeros((E,), dtype=np.float32)
    sp, ix, gl = kernel(x=x, W=W, b=b)
    print("shapes:", sp.shape, ix.shape, gl.shape, sp.dtype, ix.dtype, gl.dtype)


# revision 9
# speedup vs baseline: 1.0822x; 1.0202x over previous
"""MoE gating kernel for Trainium2 (Bass/Tile), 8-core data parallel. V2.

Structure: Wt-resident gate matmul (moving = tokens, N=512) to minimize
PE instruction count for the f32 matmul:

per core (2048 tokens), per token-group of 512 (4 tiles of 128):
  - DMA 4 x-tiles [128, 2048] (natural layout)
  - PE-transpose every [128,128] block -> PSUM -> copy to xT [128, 16, 512]
  - 16 accumulating matmuls: lhsT = Wt chunk [128h, 8e] (tiny stationary),
    rhs = xT[:, c, :] [128h, 512t] -> PSUM gateT [8, 512]
  - copy gateT -> SBUF, 4 tiny PE back-transposes -> [128, 8] per tile
  - DVE top-2-of-8: +b, max (sorts desc), max_index, mask >= 2nd max,
    sparse = where(mask, logits, -inf)
  - DMA out gate/sparse/idx tiles
"""

import sys

for _p in ("/opt/trn_rl_repo", "/root/.axon_site"):
    if _p not in sys.path:
        sys.path.insert(0, _p)

from contextlib import ExitStack

import numpy as np

import concourse.bacc as bacc
import concourse.mybir as mybir
import concourse.tile as tile
from concourse.bass_utils import run_bass_kernel_spmd

B, S, H, E, TOPK = 4, 4096, 2048, 8, 2
N_CORES = 8
TOKENS = B * S
TOK_PER_CORE = TOKENS // N_CORES  # 2048
P = 128
N_TILES = TOK_PER_CORE // P       # 16
HC = H // P                       # 16 h-chunks
GRP = 512                         # tokens per matmul group
N_GRP = TOK_PER_CORE // GRP       # 4
TPG = GRP // P                    # tiles per group = 4
F32 = mybir.dt.float32
NEG_INF = float("-inf")

_cache = {}
_EYE = np.eye(P, dtype=np.float32)


def _build():
    nc = bacc.Bacc("TRN2", target_bir_lowering=False, debug=False)

    x_d = nc.dram_tensor("x", [TOK_PER_CORE, H], F32, kind="ExternalInput")
    wt_d = nc.dram_tensor("wt", [P, HC * E], F32, kind="ExternalInput")
    bb_d = nc.dram_tensor("bb", [P, E], F32, kind="ExternalInput")
    id_d = nc.dram_tensor("ident", [P, P], F32, kind="ExternalInput")
    gate_d = nc.dram_tensor("gate", [TOK_PER_CORE, E], F32, kind="ExternalOutput")
    sp_d = nc.dram_tensor("sparse", [TOK_PER_CORE, E], F32, kind="ExternalOutput")
    idx_d = nc.dram_tensor("idx", [TOK_PER_CORE, TOPK], mybir.dt.int32,
                           kind="ExternalOutput")

    with tile.TileContext(nc) as tc:
        with ExitStack() as ctx:
            consts = ctx.enter_context(tc.tile_pool(name="consts", bufs=1))
            xpool = ctx.enter_context(tc.tile_pool(name="xin", bufs=4))
            xtg = ctx.enter_context(tc.tile_pool(name="xtg", bufs=4))
            trps = ctx.enter_context(
                tc.tile_pool(name="trps", bufs=4, space="PSUM"))
            gtps = ctx.enter_context(
                tc.tile_pool(name="gtps", bufs=2, space="PSUM"))
            btps = ctx.enter_context(
                tc.tile_pool(name="btps", bufs=2, space="PSUM"))
            gts_pool = ctx.enter_context(tc.tile_pool(name="gts", bufs=2))
            spool = ctx.enter_context(tc.tile_pool(name="small", bufs=6))

            ident = consts.tile([P, P], F32)
            nc.sync.dma_start(ident[:], id_d[:, :])
            wt_sb = consts.tile([P, HC * E], F32)
            nc.sync.dma_start(wt_sb[:], wt_d[:, :])
            bb_sb = consts.tile([P, E], F32)
            nc.sync.dma_start(bb_sb[:], bb_d[:, :])

            ncopy = 0

            def load_transpose(g):
                xT = xtg.tile([P, HC, GRP], F32, tag="xtg")
                nonlocal ncopy
                for t4 in range(TPG):
                    tt = g * TPG + t4
                    xt = xpool.tile([P, H], F32, tag="xin")
                    nc.sync.dma_start(xt[:], x_d[tt * P:(tt + 1) * P, :])
                    for gg in range(4):
                        ps = trps.tile([P, 4, P], F32, tag="trp")
                        for j in range(4):
                            c = gg * 4 + j
                            nc.tensor.transpose(
                                ps[:, j], xt[:, c * P:(c + 1) * P], ident[:])
                        dst = xT[:, gg * 4:(gg + 1) * 4, t4 * P:(t4 + 1) * P]
                        if ncopy % 2 == 0:
                            nc.vector.tensor_copy(dst, ps[:])
                        else:
                            nc.scalar.copy(dst, ps[:])
                        ncopy += 1
                return xT

            xTs = {0: load_transpose(0)}
            for g in range(N_GRP):
                if g + 1 < N_GRP:
                    xTs[g + 1] = load_transpose(g + 1)
                xT = xTs.pop(g)
                gT = gtps.tile([E, GRP], F32, tag="gt")
                for c in range(HC):
                    nc.tensor.matmul(
                        gT[:],
                        wt_sb[:, c * E:(c + 1) * E],
                        xT[:, c, :],
                        start=(c == 0),
                        stop=(c == HC - 1),
                    )
                gts = gts_pool.tile([E, GRP], F32, tag="gts")
                for q in range(TPG):
                    qs = gts[:, q * P:(q + 1) * P]
                    qp = gT[:, q * P:(q + 1) * P]
                    if q % 2 == 0:
                        nc.vector.tensor_copy(qs, qp)
                    else:
                        nc.scalar.copy(qs, qp)

                for t4 in range(TPG):
                    tt = g * TPG + t4
                    bt = btps.tile([P, E], F32, tag="bt")
                    nc.tensor.transpose(
                        bt[:], gts[:, t4 * P:(t4 + 1) * P], ident[0:E, 0:E])

                    gate = spool.tile([P, E], F32, tag="gate")
                    nc.vector.tensor_add(gate[:], bt[:], bb_sb[:])

                    mx8 = spool.tile([P, 8], F32, tag="mx8")
                    nc.vector.max(out=mx8[:], in_=gate[:])
                    ix8 = spool.tile([P, 8], mybir.dt.uint32, tag="ix8")
                    nc.vector.max_index(out=ix8[:], in_max=mx8[:],
                                        in_values=gate[:])
                    ix2 = spool.tile([P, TOPK], mybir.dt.int32, tag="ix2")
                    nc.vector.tensor_copy(ix2[:], ix8[:, 0:TOPK])

                    keep = spool.tile([P, E], mybir.dt.uint32, tag="keep")
                    nc.vector.tensor_scalar(
                        out=keep[:], in0=gate[:], scalar1=mx8[:, 1:2],
                        scalar2=None, op0=mybir.AluOpType.is_ge)
                    sp = spool.tile([P, E], F32, tag="sp")
                    nc.vector.memset(sp[:], NEG_INF)
                    nc.vector.copy_predicated(sp[:], keep[:], gate[:])

                    nc.scalar.dma_start(gate_d[tt * P:(tt + 1) * P, :], gate[:])
                    nc.scalar.dma_start(sp_d[tt * P:(tt + 1) * P, :], sp[:])
                    nc.scalar.dma_start(idx_d[tt * P:(tt + 1) * P, :], ix2[:])

    nc.compile()
    return nc


def _get_nc():
    if "nc" not in _cache:
        _cache["nc"] = _build()
    return _cache["nc"]


def _make_in_maps(x, W, b):
    x = np.ascontiguousarray(np.asarray(x, dtype=np.float32)).reshape(TOKENS, H)
    W = np.asarray(W, dtype=np.float32)
    b = np.asarray(b, dtype=np.float32)

    # Wt packed: [p, c*8+e] = W[e, c*128+p]
    wt = np.ascontiguousarray(
        W.T.reshape(HC, P, E).transpose(1, 0, 2).reshape(P, HC * E))
    bb = np.ascontiguousarray(np.broadcast_to(b, (P, E)))

    return [
        {"x": x[i * TOK_PER_CORE:(i + 1) * TOK_PER_CORE], "wt": wt, "bb": bb,
         "ident": _EYE}
        for i in range(N_CORES)
    ]


def kernel(x: np.ndarray, W: np.ndarray, b: np.ndarray):
    in_maps = _make_in_maps(x, W, b)
    nc = _get_nc()
    res = run_bass_kernel_spmd(nc, in_maps, list(range(N_CORES)))

    gate = np.concatenate([r["gate"] for r in res.results], axis=0)
    sparse = np.concatenate([r["sparse"] for r in res.results], axis=0)
    idx = np.concatenate([r["idx"] for r in res.results], axis=0)

    return (
        sparse.reshape(B, S, E),
        idx.reshape(B, S, TOPK).astype(np.int32),
        gate.reshape(TOKENS, E),
    )


if __name__ == "__main__":
    rng = np.random.default_rng(0)
    x = rng.standard_normal((B, S, H), dtype=np.float32)
    W = (rng.standard_normal((E, H), dtype=np.float32) / np.sqrt(H)).astype(
        np.float32)
    b = np.zeros((E,), dtype=np.float32)
    sp, ix, gl = kernel(x=x, W=W, b=b)
    print("shapes:", sp.shape, ix.shape, gl.shape, sp.dtype, ix.dtype, gl.dtype)
